# revision 7
# baseline (speedup 1.0000x reference)
"""CTC loss (mean reduction) on 8 Trainium2 NeuronCores.

Data-parallel over batch: 4 utterances per core, one partition each, with the
S=257 extended-label states on the free axis. The lattice DP runs t-major in
the linear-probability domain in fp32:

    A_t[s] = (A_{t-1}[s] + e^{-g} A_{t-1}[s-1] + m3[s] e^{-2g} A_{t-1}[s-2]) * p_t[s]

Range control needs no host-side DP oracle:
  * emissions are shipped as fp8 (e4m3) of exp(E + SHIFT_b), SHIFT_b a
    per-utterance constant;
  * an exact per-utterance "tilt" e^{-g*s} (g from a fitted function of the
    advance rate sl/il) is folded into the transition weights so the renorm
    max tracks the answer diagonal;
  * the device renormalizes by 1/max every RN=8 steps (reciprocal factors are
    shipped back and log-summed on the host).
Each utterance's emission stream is padded past t=il-1 with a "freeze"
pattern (p[sl]=1, else 0) whose first step computes alpha[sl]+alpha[sl-1]
exactly -- the CTC log-likelihood lands in the final alpha, so no mid-stream
snapshot is needed.

Engines: activation does all DMAs + fp8->f32 upcasts (fully unrolled, static
offsets); vector runs the DP with a hardware Fori loop over chunk pairs
(32 time steps per chunk), double-buffered via semaphores.
"""

import numpy as np
import ml_dtypes

import concourse.bass as bass
import concourse.mybir as mybir
from concourse.bass_utils import run_bass_kernel_spmd

B, T, C, U = 32, 1000, 1024, 128
S = 2 * U + 1            # 257 extended states
NCORES = 8
BPC = B // NCORES        # 4 utterances per core
CT = 32                  # time steps per chunk
NCH = 33                 # chunks (odd; chunk 0 unrolled, rest looped in pairs)
TP = NCH * CT            # padded time 1056
NIT = (NCH - 1) // 2     # Fori iterations
RN = 8                   # renorm period (steps)
CW = CT * S              # chunk width in elements 8224
OB = 260                 # outd column where renorm factors start
NRF = 4 * NCH            # total renorm factors 132
OUTW = OB + 4 + 8 * NIT  # 392
F32 = mybir.dt.float32
F8 = mybir.dt.float8e4
F8NP = mybir.dt.np(F8)
OP = mybir.AluOpType
AX = mybir.AxisListType
# tilt fit: g = polyval(GCO, sl/il), calibrated on the input distribution
GCO = (4.0775, -6.8982, 3.1779)


def _build_nc(detect_races=True):
    nc = bass.Bass(detect_race_conditions=detect_races)
    pt = nc.declare_dram_parameter("pt", [BPC, TP * S], F8, isOutput=False)
    m3eg = nc.declare_dram_parameter("m3eg", [BPC, S + 1], F32, isOutput=False)
    outd = nc.declare_dram_parameter("outd", [BPC, OUTW], F32, isOutput=True)

    with (
        nc.semaphore("s_in") as s_in,
        nc.semaphore("s_upc") as s_upc,
        nc.semaphore("s_free") as s_free,
        nc.semaphore("s_out") as s_out,
        nc.sbuf_tensor("t8A", [BPC, CW], F8) as t8A,
        nc.sbuf_tensor("t8B", [BPC, CW], F8) as t8B,
        nc.sbuf_tensor("fA", [BPC, CW], F32) as fA,
        nc.sbuf_tensor("fB", [BPC, CW], F32) as fB,
        nc.sbuf_tensor("M3", [BPC, S + 1], F32) as M3,
        nc.sbuf_tensor("AE", [BPC, S + 2], F32) as AE,
        nc.sbuf_tensor("AO", [BPC, S + 2], F32) as AO,
        nc.sbuf_tensor("s1t", [BPC, S], F32) as s1t,
        nc.sbuf_tensor("a3t", [BPC, S], F32) as a3t,
        nc.sbuf_tensor("s2t", [BPC, S], F32) as s2t,
        nc.sbuf_tensor("mtmp", [BPC, 1], F32) as mtmp,
        nc.sbuf_tensor("stg", [BPC, 8], F32) as stg,
        nc.sbuf_tensor("stg0", [BPC, 4], F32) as stg0,
    ):
        sy = nc.sync
        vec = nc.vector

        def chunk_ap(c):
            return pt[:, c * CW : (c + 1) * CW]

        # ------------- sync engine: all DMAs (unrolled, static) ------------
        sy.dma_start(out=M3[:, :], in_=m3eg[:, :]).then_inc(s_in, 16)     # 1
        sy.dma_start(out=t8B[:, :], in_=chunk_ap(0)).then_inc(s_in, 16)   # 2
        sy.dma_start(out=t8A[:, :], in_=chunk_ap(1)).then_inc(s_in, 16)   # 3
        for k in range(2, NCH):
            t8X = t8A if k % 2 else t8B
            sy.wait_ge(s_upc, k - 1)      # upcast of c_{k-2} freed t8X
            sy.dma_start(out=t8X[:, :], in_=chunk_ap(k)).then_inc(s_in, 16)
            if k == 2:
                sy.wait_ge(s_free, 1)
                sy.dma_start(out=outd[:, OB : OB + 4], in_=stg0[:, :]).then_inc(
                    s_out, 16
                )
            if k >= 4 and k % 2 == 0:
                i = (k - 4) // 2
                sy.wait_ge(s_free, k - 1)  # vector done iter i (chunks <= k-2)
                sy.dma_start(
                    out=outd[:, OB + 4 + 8 * i : OB + 12 + 8 * i], in_=stg[:, :]
                ).then_inc(s_out, 16)
        sy.wait_ge(s_free, NCH)
        sy.dma_start(
            out=outd[:, OB + 4 + 8 * (NIT - 1) : OUTW], in_=stg[:, :]
        ).then_inc(s_out, 16)
        sy.dma_start(out=outd[:, 0 : S + 2], in_=AO[:, :]).then_inc(s_out, 16)
        sy.wait_ge(s_out, 16 * (NIT + 2))

        # ---------------- vector engine: upcasts + the DP ------------------
        # Deferred renorm: the boundary after step lt (lt%8==7) reduces the
        # max into mtmp; the NEXT step computes 1/max into its stage slot and
        # folds the scale into its emission multiply. The last boundary of
        # the run is neither applied nor shipped. All short-op reads sit >=1
        # instruction behind their producer (DVE gap-0 in1 hazard).
        def step(src, dst, pf, lt, slot_ap):
            vec.tensor_tensor(a3t[:, :], src[:, 0:S], M3[:, 0:S], OP.mult)
            if slot_ap is not None and lt % RN == 0:
                vec.reciprocal(slot_ap, mtmp[:, :])
            vec.scalar_tensor_tensor(
                s1t[:, :], src[:, 1 : 1 + S], M3[:, S : S + 1],
                src[:, 2 : 2 + S], OP.mult, OP.add,
            )
            vec.tensor_tensor(s2t[:, :], s1t[:, :], a3t[:, :], OP.add)
            pslice = pf[:, lt * S : (lt + 1) * S]
            if slot_ap is not None and lt % RN == 0:
                last = vec.scalar_tensor_tensor(
                    dst[:, 2 : 2 + S], s2t[:, :], slot_ap, pslice,
                    OP.mult, OP.mult,
                )
            else:
                last = vec.tensor_tensor(
                    dst[:, 2 : 2 + S], s2t[:, :], pslice, OP.mult
                )
            if lt % RN == RN - 1:
                last = vec.tensor_reduce(
                    mtmp[:, :], dst[:, 2 : 2 + S], AX.X, OP.max
                )
            return last

        def pslice_ap(pf, lt):
            return pf[:, lt * S : (lt + 1) * S]

        # guards stay zero forever; AE body is re-zeroed where the t=0 init
        # does not write; AO body is fully written by the first step.
        vec.memset(AE[:, 0:2], 0.0)
        vec.memset(AO[:, 0:2], 0.0)
        vec.memset(AE[:, 4 : S + 2], 0.0)
        vec.wait_ge(s_in, 32)                     # M3 + c0 landed
        vec.tensor_copy(fB[:, :], t8B[:, :]).then_inc(s_upc, 1)   # upcast c0
        vec.tensor_copy(AE[:, 2:4], fB[:, 0:2])   # t=0 init (tilt pre-baked)
        vec.memset(stg0[:, 3:4], 1.0)             # dummy factor (log == 0)
        last = None
        for lt in range(1, CT):                   # chunk 0: steps 1..31
            src, dst = (AO, AE) if lt % 2 == 0 else (AE, AO)
            slot = stg0[:, lt // RN - 1 : lt // RN] if lt % RN == 0 else None
            last = step(src, dst, fB, lt, slot)
        last.then_inc(s_free, 1)

        rI = vec.alloc_register("rI")
        rO = vec.alloc_register("rO")
        vec.reg_mov(rI, 32)
        vec.reg_mov(rO, 0)
        with vec.Fori(0, NIT):
            vec.reg_add(rO, rO, 16)
            vec.wait_ge(s_out, rO)                # stage DMA of prev iter done
            for half, (t8X, fX) in ((0, (t8A, fA)), (1, (t8B, fB))):
                vec.reg_add(rI, rI, 16)
                vec.wait_ge(s_in, rI)             # this chunk's DMA landed
                vec.tensor_copy(fX[:, :], t8X[:, :]).then_inc(s_upc, 1)
                base = 4 * half
                last = None
                for lt in range(CT):
                    src, dst = (AO, AE) if lt % 2 == 0 else (AE, AO)
                    if lt % RN == 0:
                        c = (base + lt // RN - 1) % 8
                        slot = stg[:, c : c + 1]
                    else:
                        slot = None
                    last = step(src, dst, fX, lt, slot)
                last.then_inc(s_free, 1)

    return nc


_NC_CACHE = None
_LAST_IN_MAPS = None


def _prep(lp, tg, il, tl):
    """Host-side emission prep. Returns (in_maps, g, shift, sl)."""
    ext = np.zeros((B, S), np.int32)
    ext[:, 1::2] = tg
    prev2 = np.concatenate([np.zeros((B, 2), np.int32), ext[:, :-2]], axis=1)
    m3 = ((ext != 0) & (ext != prev2)).astype(np.float32)
    E = np.take_along_axis(lp, ext[:, None, :], axis=2)      # [B,T,S] f32
    sl = (2 * tl).astype(np.int64)

    nu = sl / il
    g = np.polyval(GCO, nu)
    g = np.clip(g, 0.2, 3.5).astype(np.float64)

    # per-utterance shift so exp(E + shift) fits fp8 e4m3 (max ~240)
    Emax = E.max(axis=(1, 2)).astype(np.float64)
    shift = np.minimum(7.5, 5.0 - Emax)

    p8 = np.zeros((B, TP, S), F8NP)
    for b in range(B):
        ib = int(il[b])
        pf = np.exp(E[b, :ib].astype(np.float64) + shift[b])
        pf[0, 1] *= np.exp(-g[b])          # tilt on the t=0 init of state 1
        p8[b, :ib] = np.minimum(pf, 224.0).astype(F8NP)
        p8[b, ib:, sl[b]] = 1.0            # freeze pattern
    m3eg = np.zeros((B, S + 1), np.float32)
    m3eg[:, :S] = m3 * np.exp(-2 * g)[:, None]
    m3eg[:, S] = np.exp(-g)

    in_maps = []
    for c in range(NCORES):
        bs = slice(c * BPC, (c + 1) * BPC)
        in_maps.append({
            "pt": np.ascontiguousarray(p8[bs].reshape(BPC, TP * S)),
            "m3eg": np.ascontiguousarray(m3eg[bs]),
        })
    return in_maps, g, shift, sl, ext, m3


def _ll_exact(lp, ext, m3, il, sl, bsel):
    """Float64 log-domain DP fallback for utterances in bsel."""
    nb = len(bsel)
    E = np.take_along_axis(
        lp[bsel].astype(np.float64), ext[bsel][:, None, :], axis=2)
    NEGL = -1e30
    a = np.full((nb, S), NEGL)
    a[:, 0] = E[:, 0, 0]
    a[:, 1] = E[:, 0, 1]
    m3b = m3[bsel] > 0
    snap = np.zeros((nb, S))
    ilb = il[bsel]
    for t in range(int(ilb.max())):
        if t > 0:
            a2 = np.concatenate([np.full((nb, 1), NEGL), a[:, :-1]], axis=1)
            a3 = np.where(
                m3b,
                np.concatenate([np.full((nb, 2), NEGL), a[:, :-2]], axis=1),
                NEGL,
            )
            m = np.maximum(np.maximum(a, a2), a3)
            a = m + np.log(
                np.exp(a - m) + np.exp(a2 - m) + np.exp(a3 - m)
            ) + E[:, t, :]
        hit = (ilb - 1) == t
        if hit.any():
            snap[hit] = a[hit]
    slb = sl[bsel]
    r = np.arange(nb)
    return np.logaddexp(snap[r, slb], snap[r, slb - 1])


def kernel(log_probs, targets, input_lengths, target_lengths):
    global _NC_CACHE, _LAST_IN_MAPS
    lp = np.asarray(log_probs, np.float32)
    tg = np.asarray(targets, np.int32)
    il = np.asarray(input_lengths, np.int64)
    tl = np.asarray(target_lengths, np.int64)

    in_maps, g, shift, sl, ext, m3 = _prep(lp, tg, il, tl)
    if _NC_CACHE is None:
        _NC_CACHE = _build_nc()
    _LAST_IN_MAPS = in_maps
    res = run_bass_kernel_spmd(_NC_CACHE, in_maps, core_ids=list(range(NCORES)))

    ll = np.zeros(B, np.float64)
    bad = []
    for b in range(B):
        core, row = b // BPC, b % BPC
        o = res.results[core]["outd"][row].astype(np.float64)
        afin = o[2 + sl[b]]
        rhat = o[OB:OUTW]
        if afin > 0 and np.all(rhat > 0) and np.all(np.isfinite(rhat)):
            ll[b] = (np.log(afin) - np.log(rhat).sum()
                     - shift[b] * il[b] + g[b] * sl[b])
        else:
            bad.append(b)
    if bad:
        ll[bad] = _ll_exact(lp, ext, m3, il, sl, np.array(bad))
    loss = -ll.sum() / il.sum()
    return np.float32(loss)


# revision 8
# speedup vs baseline: 1.9727x; 1.9727x over previous
"""CTC loss (mean reduction) on 8 Trainium2 NeuronCores.

Data-parallel over batch: 4 utterances per core, one partition each, with the
S=257 extended-label states on the free axis. The lattice DP runs t-major in
the linear-probability domain in fp32:

    A_t[s] = (A_{t-1}[s] + e^{-g} A_{t-1}[s-1] + m3[s] e^{-2g} A_{t-1}[s-2]) * p_t[s]

Range control needs no host-side DP oracle:
  * emissions are shipped as fp8 (e4m3) of exp(E + SHIFT_b), SHIFT_b a
    per-utterance constant;
  * an exact per-utterance "tilt" e^{-g*s} (g from a fitted function of the
    advance rate sl/il) is folded into the transition weights so the renorm
    max tracks the answer diagonal;
  * the device renormalizes by 1/max every RN=8 steps (reciprocal factors are
    shipped back and log-summed on the host).
Each utterance's emission stream is padded past t=il-1 with a "freeze"
pattern (p[sl]=1, else 0) whose first step computes alpha[sl]+alpha[sl-1]
exactly -- the CTC log-likelihood lands in the final alpha, so no mid-stream
snapshot is needed.

Engines: activation does all DMAs + fp8->f32 upcasts (fully unrolled, static
offsets); vector runs the DP with a hardware Fori loop over chunk pairs
(32 time steps per chunk), double-buffered via semaphores.
"""

import numpy as np
import ml_dtypes

import concourse.bass as bass
import concourse.mybir as mybir
from concourse.bass_utils import run_bass_kernel_spmd

B, T, C, U = 32, 1000, 1024, 128
S = 2 * U + 1            # 257 extended states
NCORES = 8
BPC = B // NCORES        # 4 utterances per core
CT = 32                  # time steps per chunk
NCH = 33                 # chunks (odd; chunk 0 unrolled, rest looped in pairs)
TP = NCH * CT            # padded time 1056
NIT = (NCH - 1) // 2     # Fori iterations
RN = 8                   # renorm period (steps)
CW = CT * S              # chunk width in elements 8224
OB = 260                 # outd column where renorm factors start
NRF = 4 * NCH            # total renorm factors 132
OUTW = OB + 4 + 8 * NIT  # 392
F32 = mybir.dt.float32
F8 = mybir.dt.float8e4
F8NP = mybir.dt.np(F8)
OP = mybir.AluOpType
AX = mybir.AxisListType
# tilt fit: g = polyval(GCO, sl/il), calibrated on the input distribution
GCO = (4.0775, -6.8982, 3.1779)


def _build_nc(detect_races=True):
    nc = bass.Bass(detect_race_conditions=detect_races)
    pt = nc.declare_dram_parameter("pt", [BPC, TP * S], F8, isOutput=False)
    m3eg = nc.declare_dram_parameter("m3eg", [BPC, S + 1], F32, isOutput=False)
    outd = nc.declare_dram_parameter("outd", [BPC, OUTW], F32, isOutput=True)

    with (
        nc.semaphore("s_in") as s_in,
        nc.semaphore("s_upc") as s_upc,
        nc.semaphore("s_free") as s_free,
        nc.semaphore("s_out") as s_out,
        nc.sbuf_tensor("t8A", [BPC, CW], F8) as t8A,
        nc.sbuf_tensor("t8B", [BPC, CW], F8) as t8B,
        nc.sbuf_tensor("fA", [BPC, CW], F32) as fA,
        nc.sbuf_tensor("fB", [BPC, CW], F32) as fB,
        nc.sbuf_tensor("M3", [BPC, S + 1], F32) as M3,
        nc.sbuf_tensor("AE", [BPC, S + 2], F32) as AE,
        nc.sbuf_tensor("AO", [BPC, S + 2], F32) as AO,
        nc.sbuf_tensor("s1t", [BPC, S], F32) as s1t,
        nc.sbuf_tensor("a3t", [BPC, S], F32) as a3t,
        nc.sbuf_tensor("s2t", [BPC, S], F32) as s2t,
        nc.sbuf_tensor("mtmp", [BPC, 1], F32) as mtmp,
        nc.sbuf_tensor("stg", [BPC, 8], F32) as stg,
        nc.sbuf_tensor("stg0", [BPC, 4], F32) as stg0,
    ):
        sy = nc.sync
        vec = nc.vector

        def chunk_ap(c):
            return pt[:, c * CW : (c + 1) * CW]

        # ------------- sync engine: all DMAs (unrolled, static) ------------
        sy.dma_start(out=M3[:, :], in_=m3eg[:, :]).then_inc(s_in, 16)     # 1
        sy.dma_start(out=t8B[:, :], in_=chunk_ap(0)).then_inc(s_in, 16)   # 2
        sy.dma_start(out=t8A[:, :], in_=chunk_ap(1)).then_inc(s_in, 16)   # 3
        for k in range(2, NCH):
            t8X = t8A if k % 2 else t8B
            sy.wait_ge(s_upc, k - 1)      # upcast of c_{k-2} freed t8X
            sy.dma_start(out=t8X[:, :], in_=chunk_ap(k)).then_inc(s_in, 16)
            if k == 2:
                sy.wait_ge(s_free, 1)
                sy.dma_start(out=outd[:, OB : OB + 4], in_=stg0[:, :]).then_inc(
                    s_out, 16
                )
            if k >= 4 and k % 2 == 0:
                i = (k - 4) // 2
                sy.wait_ge(s_free, k - 1)  # vector done iter i (chunks <= k-2)
                sy.dma_start(
                    out=outd[:, OB + 4 + 8 * i : OB + 12 + 8 * i], in_=stg[:, :]
                ).then_inc(s_out, 16)
        sy.wait_ge(s_free, NCH)
        sy.dma_start(
            out=outd[:, OB + 4 + 8 * (NIT - 1) : OUTW], in_=stg[:, :]
        ).then_inc(s_out, 16)
        sy.dma_start(out=outd[:, 0 : S + 2], in_=AO[:, :]).then_inc(s_out, 16)
        sy.wait_ge(s_out, 16 * (NIT + 2))

        # ---------------- vector engine: upcasts + the DP ------------------
        # Deferred renorm: the boundary step (lt%8==7) sums its output row
        # into mtmp via accum_out (tensor_reduce with free size > 16 silently
        # no-ops on this HW); the NEXT step computes 1/sum into its stage
        # slot and folds the scale into its emission multiply. The last
        # boundary of the run is neither applied nor shipped. All short-op
        # reads sit >=1 instruction behind their producer (DVE gap-0 in1
        # hazard on short ops).
        def step(src, dst, pf, lt, slot_ap):
            vec.tensor_tensor(a3t[:, :], src[:, 0:S], M3[:, 0:S], OP.mult)
            if slot_ap is not None and lt % RN == 0:
                vec.reciprocal(slot_ap, mtmp[:, :])
            vec.scalar_tensor_tensor(
                s1t[:, :], src[:, 1 : 1 + S], M3[:, S : S + 1],
                src[:, 2 : 2 + S], OP.mult, OP.add,
            )
            vec.tensor_tensor(s2t[:, :], s1t[:, :], a3t[:, :], OP.add)
            pslice = pf[:, lt * S : (lt + 1) * S]
            if slot_ap is not None and lt % RN == 0:
                last = vec.scalar_tensor_tensor(
                    dst[:, 2 : 2 + S], s2t[:, :], slot_ap, pslice,
                    OP.mult, OP.mult,
                )
            elif lt % RN == RN - 1:
                last = vec.scalar_tensor_tensor(
                    dst[:, 2 : 2 + S], s2t[:, :], 1.0, pslice,
                    OP.mult, OP.mult, accum_out=mtmp[:, :],
                )
            else:
                last = vec.tensor_tensor(
                    dst[:, 2 : 2 + S], s2t[:, :], pslice, OP.mult
                )
            return last

        def pslice_ap(pf, lt):
            return pf[:, lt * S : (lt + 1) * S]

        # guards stay zero forever; AE body is re-zeroed where the t=0 init
        # does not write; AO body is fully written by the first step.
        vec.memset(AE[:, 0:2], 0.0)
        vec.memset(AO[:, 0:2], 0.0)
        vec.memset(AE[:, 4 : S + 2], 0.0)
        vec.wait_ge(s_in, 32)                     # M3 + c0 landed
        vec.tensor_copy(fB[:, :], t8B[:, :]).then_inc(s_upc, 1)   # upcast c0
        vec.tensor_copy(AE[:, 2:4], fB[:, 0:2])   # t=0 init (tilt pre-baked)
        vec.memset(stg0[:, 3:4], 1.0)             # dummy factor (log == 0)
        last = None
        for lt in range(1, CT):                   # chunk 0: steps 1..31
            src, dst = (AO, AE) if lt % 2 == 0 else (AE, AO)
            slot = stg0[:, lt // RN - 1 : lt // RN] if lt % RN == 0 else None
            last = step(src, dst, fB, lt, slot)
        last.then_inc(s_free, 1)

        rI = vec.alloc_register("rI")
        rO = vec.alloc_register("rO")
        vec.reg_mov(rI, 32)
        vec.reg_mov(rO, 0)
        with vec.Fori(0, NIT):
            vec.reg_add(rO, rO, 16)
            vec.wait_ge(s_out, rO)                # stage DMA of prev iter done
            for half, (t8X, fX) in ((0, (t8A, fA)), (1, (t8B, fB))):
                vec.reg_add(rI, rI, 16)
                vec.wait_ge(s_in, rI)             # this chunk's DMA landed
                vec.tensor_copy(fX[:, :], t8X[:, :]).then_inc(s_upc, 1)
                base = 4 * half
                last = None
                for lt in range(CT):
                    src, dst = (AO, AE) if lt % 2 == 0 else (AE, AO)
                    if lt % RN == 0:
                        c = (base + lt // RN - 1) % 8
                        slot = stg[:, c : c + 1]
                    else:
                        slot = None
                    last = step(src, dst, fX, lt, slot)
                last.then_inc(s_free, 1)

    return nc


_NC_CACHE = None
_LAST_IN_MAPS = None


def _prep(lp, tg, il, tl):
    """Host-side emission prep. Returns (in_maps, g, shift, sl)."""
    ext = np.zeros((B, S), np.int32)
    ext[:, 1::2] = tg
    prev2 = np.concatenate([np.zeros((B, 2), np.int32), ext[:, :-2]], axis=1)
    m3 = ((ext != 0) & (ext != prev2)).astype(np.float32)
    E = np.take_along_axis(lp, ext[:, None, :], axis=2)      # [B,T,S] f32
    sl = (2 * tl).astype(np.int64)

    nu = sl / il
    g = np.polyval(GCO, nu)
    g = np.clip(g, 0.2, 3.5).astype(np.float64)

    # per-utterance shift so exp(E + shift) fits fp8 e4m3 (max ~240)
    Emax = E.max(axis=(1, 2)).astype(np.float64)
    shift = np.minimum(7.5, 5.0 - Emax)

    p8 = np.zeros((B, TP, S), F8NP)
    for b in range(B):
        ib = int(il[b])
        pf = np.exp(E[b, :ib].astype(np.float64) + shift[b])
        pf[0, 1] *= np.exp(-g[b])          # tilt on the t=0 init of state 1
        p8[b, :ib] = np.minimum(pf, 224.0).astype(F8NP)
        p8[b, ib:, sl[b]] = 1.0            # freeze pattern
    m3eg = np.zeros((B, S + 1), np.float32)
    m3eg[:, :S] = m3 * np.exp(-2 * g)[:, None]
    m3eg[:, S] = np.exp(-g)

    in_maps = []
    for c in range(NCORES):
        bs = slice(c * BPC, (c + 1) * BPC)
        in_maps.append({
            "pt": np.ascontiguousarray(p8[bs].reshape(BPC, TP * S)),
            "m3eg": np.ascontiguousarray(m3eg[bs]),
        })
    return in_maps, g, shift, sl, ext, m3


def _ll_exact(lp, ext, m3, il, sl, bsel):
    """Float64 log-domain DP fallback for utterances in bsel."""
    nb = len(bsel)
    E = np.take_along_axis(
        lp[bsel].astype(np.float64), ext[bsel][:, None, :], axis=2)
    NEGL = -1e30
    a = np.full((nb, S), NEGL)
    a[:, 0] = E[:, 0, 0]
    a[:, 1] = E[:, 0, 1]
    m3b = m3[bsel] > 0
    snap = np.zeros((nb, S))
    ilb = il[bsel]
    for t in range(int(ilb.max())):
        if t > 0:
            a2 = np.concatenate([np.full((nb, 1), NEGL), a[:, :-1]], axis=1)
            a3 = np.where(
                m3b,
                np.concatenate([np.full((nb, 2), NEGL), a[:, :-2]], axis=1),
                NEGL,
            )
            m = np.maximum(np.maximum(a, a2), a3)
            a = m + np.log(
                np.exp(a - m) + np.exp(a2 - m) + np.exp(a3 - m)
            ) + E[:, t, :]
        hit = (ilb - 1) == t
        if hit.any():
            snap[hit] = a[hit]
    slb = sl[bsel]
    r = np.arange(nb)
    return np.logaddexp(snap[r, slb], snap[r, slb - 1])


def kernel(log_probs, targets, input_lengths, target_lengths):
    global _NC_CACHE, _LAST_IN_MAPS
    lp = np.asarray(log_probs, np.float32)
    tg = np.asarray(targets, np.int32)
    il = np.asarray(input_lengths, np.int64)
    tl = np.asarray(target_lengths, np.int64)

    in_maps, g, shift, sl, ext, m3 = _prep(lp, tg, il, tl)
    if _NC_CACHE is None:
        _NC_CACHE = _build_nc()
    _LAST_IN_MAPS = in_maps
    res = run_bass_kernel_spmd(_NC_CACHE, in_maps, core_ids=list(range(NCORES)))

    ll = np.zeros(B, np.float64)
    bad = []
    for b in range(B):
        core, row = b // BPC, b % BPC
        o = res.results[core]["outd"][row].astype(np.float64)
        afin = o[2 + sl[b]]
        rhat = o[OB:OUTW]
        if afin > 0 and np.all(rhat > 0) and np.all(np.isfinite(rhat)):
            ll[b] = (np.log(afin) - np.log(rhat).sum()
                     - shift[b] * il[b] + g[b] * sl[b])
        else:
            bad.append(b)
    if bad:
        ll[bad] = _ll_exact(lp, ext, m3, il, sl, np.array(bad))
    loss = -ll.sum() / il.sum()
    return np.float32(loss)
